# revision 9
# baseline (speedup 1.0000x reference)
"""Trainium2 Bass kernel for the Diversity8 loss.

loss = SCALE * mean_b d[b],   d[b] = (||sum_m v_m[b]||^2 - M) / 2
where v_m[b] = unit-normalized, mean-centered softmax(logits_m[b]/T).

Softmax centering+normalization is shift/scale invariant, so v_m is the
unit-normalized centered e_m = exp(x_m/T).  Per 128-sample group we
compute, for each of the 8 models:
  - e = Exp(x/T) on ACT (accum_out gives S = sum e for free)
  - M2 = sum (e-eb)^2 = sum (e-eb)*e  via one fused pass
    (ACT Square(e - eb) for some models, DVE scalar_tensor_tensor for the
    rest -- split for engine balance)
  - alpha = rsqrt(M2) via Newton iterations on DVE (keeps ACT on a single
    exp_and_others table set)
  - u += diag(alpha_m) @ e_m accumulated in PSUM via PE matmuls (float32r)
then d = 0.5*(sum u^2 - 2k sum u + C k^2) - M/2 with k = sum_m alpha_m*ebar_m.

Sharding: pure data parallel over the batch dim, 512 samples per core on
8 cores; host sums the per-core [128, 4] d-columns.
"""

import os
import sys

import numpy as np

for _p in ("/opt/trn_rl_repo", "/root/.axon_site/_ro/trn_rl_repo"):
    if os.path.isdir(_p) and _p not in sys.path:
        sys.path.append(_p)

import concourse.bacc as bacc
import concourse.mybir as mybir
from concourse import bass_utils
from concourse.tile import TileContext

F32 = mybir.dt.float32
F32R = mybir.dt.float32r
I32 = mybir.dt.int32
AF = mybir.ActivationFunctionType
OP = mybir.AluOpType

B = 4096
C = 1000
M = 8
T = 20.0
SCALE = 0.3
N_CORES = 8
B_SHARD = B // N_CORES          # 512 samples per core
G = B_SHARD // 128              # 4 groups of 128 samples
NEWTON_ITERS = 5
RSQRT_SEED = 0.6324555          # ~ 1/sqrt(2.5), the expected M2
# models whose variance pass runs on ACT (rest on DVE) -- engine balance
ACT_VAR_MODELS = (0,)
PSUM_SPLITS = ((0, 512), (512, 1000))
MM_DT = mybir.dt.float32r        # matmul operand dtype (float32r = full-speed fp32)

_cached = {}


def _build():
    nc = bacc.Bacc("TRN2", target_bir_lowering=False, debug=False)
    xs = [
        nc.dram_tensor(f"x{m}", [B_SHARD, C], F32, kind="ExternalInput")
        for m in range(M)
    ]
    d_dram = nc.dram_tensor("d", [128, G], F32, kind="ExternalOutput")

    with TileContext(nc) as tc:
        with (
            tc.tile_pool(name="const", bufs=1) as const_pool,
            tc.tile_pool(name="x", bufs=3) as x_pool,
            tc.tile_pool(name="e", bufs=3) as e_pool,
            tc.tile_pool(name="ec", bufs=12) as ec_pool,
            tc.tile_pool(name="scr", bufs=3) as scr_pool,
            tc.tile_pool(name="stat", bufs=2) as stat_pool,
            tc.tile_pool(name="w", bufs=3) as w_pool,
            tc.tile_pool(name="dout", bufs=1) as dout_pool,
            tc.tile_pool(name="psum", bufs=2, space="PSUM") as psum_pool,
        ):
            # identity matrix: I[p, j] = (j == p); 0..127 are exact in f32
            row_iota = const_pool.tile([128, 128], F32)
            nc.gpsimd.iota(row_iota[:, :], pattern=[[1, 128]], base=0,
                           channel_multiplier=0,
                           allow_small_or_imprecise_dtypes=True)
            p_iota = const_pool.tile([128, 1], F32)
            nc.gpsimd.iota(p_iota[:, :], pattern=[[0, 1]], base=0,
                           channel_multiplier=1,
                           allow_small_or_imprecise_dtypes=True)
            ident = const_pool.tile([128, 128], F32)
            nc.vector.tensor_scalar(ident[:, :], row_iota[:, :],
                                    p_iota[:, :], None, OP.is_equal)

            dout = dout_pool.tile([128, G], F32)

            for g in range(G):
                r0 = g * 128
                s_all = stat_pool.tile([128, M], F32)     # sum e per model
                negm = stat_pool.tile([128, M], F32)      # -mean e
                m2 = stat_pool.tile([128, M], F32)        # sum (e-eb)^2
                ec_tiles = []
                for m in range(M):
                    x_t = x_pool.tile([128, C], F32)
                    nc.sync.dma_start(out=x_t[:, :],
                                      in_=xs[m].ap()[r0:r0 + 128, :])
                    e_t = e_pool.tile([128, C], F32)
                    nc.scalar.activation(e_t[:, :], x_t[:, :], AF.Exp,
                                         scale=1.0 / T,
                                         accum_out=s_all[:, m:m + 1])
                    nc.vector.tensor_scalar(negm[:, m:m + 1],
                                            s_all[:, m:m + 1],
                                            -1.0 / C, None, OP.mult)
                    # centered ec = e - mean(e) on the (otherwise idle)
                    # GPSIMD engine; centering before the matmul keeps all
                    # downstream rounding noise relative to |s|~0.1 instead
                    # of |u|~5 (the chi^2 noise bias killed accuracy).
                    ec_t = ec_pool.tile([128, C], MM_DT)
                    nc.gpsimd.tensor_scalar(ec_t[:, :], e_t[:, :],
                                            negm[:, m:m + 1], None, OP.add)
                    scr = scr_pool.tile([128, C], F32)
                    if m in ACT_VAR_MODELS:
                        nc.scalar.activation(scr[:, :], ec_t[:, :], AF.Square,
                                             accum_out=m2[:, m:m + 1])
                    else:
                        nc.vector.scalar_tensor_tensor(
                            scr[:, :], ec_t[:, :], 0.0, ec_t[:, :],
                            op0=OP.bypass, op1=OP.mult,
                            accum_out=m2[:, m:m + 1])
                    ec_tiles.append(ec_t)

                # alpha = rsqrt(m2) by Newton: y <- y * (1.5 - 0.5 * m2 * y^2)
                alpha = stat_pool.tile([128, M], F32)
                tn = stat_pool.tile([128, M], F32)
                nc.vector.memset(alpha[:, :], RSQRT_SEED)
                for _ in range(NEWTON_ITERS):
                    nc.vector.tensor_tensor(tn[:, :], alpha[:, :], alpha[:, :],
                                            OP.mult)
                    nc.vector.tensor_tensor(tn[:, :], tn[:, :], m2[:, :],
                                            OP.mult)
                    nc.vector.tensor_scalar(tn[:, :], tn[:, :], -0.5, 1.5,
                                            OP.mult, OP.add)
                    nc.vector.tensor_tensor(alpha[:, :], alpha[:, :], tn[:, :],
                                            OP.mult)

                # s = sum_m diag(alpha_m) @ ec_m  (PSUM accumulation)
                s_ps = psum_pool.tile([128, C], F32)
                for m in range(M):
                    w_t = w_pool.tile([128, 128], MM_DT)
                    nc.vector.tensor_scalar(w_t[:, :], ident[:, :],
                                            alpha[:, m:m + 1], None, OP.mult)
                    for c0, c1 in PSUM_SPLITS:
                        nc.tensor.matmul(s_ps[:, c0:c1],
                                         w_t[:, :],
                                         ec_tiles[m][:, c0:c1],
                                         start=(m == 0), stop=(m == M - 1))

                # d = 0.5 * sum s^2 - M/2
                r_col = stat_pool.tile([128, 1], F32)
                scr2 = scr_pool.tile([128, C], F32)
                nc.scalar.activation(scr2[:, :], s_ps[:, :], AF.Square,
                                     accum_out=r_col[:, :])
                nc.vector.tensor_scalar(dout[:, g:g + 1], r_col[:, :],
                                        0.5, -M / 2.0, OP.mult, OP.add)

            nc.sync.dma_start(out=d_dram.ap(), in_=dout[:, :])

    nc.compile()
    return nc


def _get_nc():
    if "nc" not in _cached:
        _cached["nc"] = _build()
    return _cached["nc"]


def kernel(**inputs: np.ndarray) -> np.ndarray:
    nc = _get_nc()
    outs = [np.asarray(inputs[f"outputs{m + 1}"], dtype=np.float32)
            for m in range(M)]
    in_maps = []
    for c in range(N_CORES):
        sl = slice(c * B_SHARD, (c + 1) * B_SHARD)
        in_maps.append(
            {f"x{m}": np.ascontiguousarray(outs[m][sl]) for m in range(M)}
        )
    res = bass_utils.run_bass_kernel_spmd(nc, in_maps,
                                          core_ids=list(range(N_CORES)))
    total = 0.0
    for c in range(N_CORES):
        total += float(res.results[c]["d"].astype(np.float64).sum())
    return np.array(SCALE * total / B, dtype=np.float32)


# revision 14
# speedup vs baseline: 5.4573x; 5.4573x over previous
"""Trainium2 Bass kernel for the Diversity8 loss.

loss = SCALE * mean_b d[b],   d[b] = (||sum_m v_m[b]||^2 - M) / 2
where v_m[b] = unit-normalized, mean-centered softmax(logits_m[b]/T).

Softmax centering + normalization are shift/scale invariant, so
v_m = (e - mean e) / ||e - mean e||  with  e = exp(x/T) (any overall
scale of e drops out -- we use e' = exp(x/T)/C so the activation's
accum_out IS the mean).

Per (model m, 128-sample group), all stats per partition row:
  - ACT:  e' = Exp(x/T + ln(1/C)), accum_out -> ebar (the mean)
  - var:  m2 = sum (e'-ebar)*e' == sum (e'-ebar)^2  in one fused pass
          (split between ACT Square(e'+(-ebar)) and DVE
          scalar_tensor_tensor for engine balance)
  - DVE:  alpha = rsqrt(m2) via Newton iterations (no ACT table switch)
  - DVE:  ecs = (e' - ebar) * alpha  in ONE 2-scalar tensor_scalar
          (2x perf mode), written as float32r
  - PE :  s += I @ ecs_m accumulated over models in PSUM; identity
          weights make every product exact, so PE adds no multiply noise
  - ACT:  R = sum s^2 via Square with accum_out;  d = 0.5*R - M/2
Centering BEFORE the matmul is load-bearing for accuracy: the loss is a
near-cancelling mean, and any elementwise noise delta on s biases d by
C*var(delta) (chi^2), so all rounding must be relative to |s|~0.1.

Sharding: pure data parallel over the batch dim, 512 samples per core on
8 cores; host sums the per-core [128, 4] d-columns.
"""

import math
import os
import sys

import numpy as np

for _p in ("/opt/trn_rl_repo", "/root/.axon_site/_ro/trn_rl_repo"):
    if os.path.isdir(_p) and _p not in sys.path:
        sys.path.append(_p)

import concourse.bacc as bacc
import concourse.mybir as mybir
from concourse import bass_utils
from concourse.tile import TileContext

F32 = mybir.dt.float32
F32R = mybir.dt.float32r
AF = mybir.ActivationFunctionType
OP = mybir.AluOpType

B = 4096
C = 1000
M = 8
T = 20.0
SCALE = 0.3
N_CORES = 8
B_SHARD = B // N_CORES          # 512 samples per core
G = B_SHARD // 128              # 4 groups of 128 samples
NEWTON_ITERS = 4
# alpha = 1/sqrt(m2), m2 ~ 2.5/C^2 concentrated within ~+-20%
RSQRT_SEED = 0.6324555
# models whose variance pass runs on ACT (rest on DVE) -- engine balance
ACT_VAR_MODELS = (0, 1, 2)
PSUM_SPLITS = ((0, 512), (512, 1000))
MM_DT = F32R                    # matmul operand dtype (full-speed fp32)

_cached = {}


def _build():
    nc = bacc.Bacc("TRN2", target_bir_lowering=False, debug=False)
    xs = [
        nc.dram_tensor(f"x{m}", [B_SHARD, C], F32, kind="ExternalInput")
        for m in range(M)
    ]
    d_dram = nc.dram_tensor("d", [128, G], F32, kind="ExternalOutput")

    with TileContext(nc) as tc:
        with (
            tc.tile_pool(name="const", bufs=1) as const_pool,
            tc.tile_pool(name="x", bufs=4) as x_pool,
            tc.tile_pool(name="e", bufs=12) as e_pool,
            tc.tile_pool(name="ec", bufs=12) as ec_pool,
            tc.tile_pool(name="w", bufs=3) as w_pool,
            tc.tile_pool(name="scr", bufs=3) as scr_pool,
            tc.tile_pool(name="stat", bufs=2) as stat_pool,
            tc.tile_pool(name="dout", bufs=1) as dout_pool,
            tc.tile_pool(name="psum", bufs=2, space="PSUM") as psum_pool,
        ):
            # identity matrix in float32r: I[p, j] = (j == p)
            row_iota = const_pool.tile([128, 128], F32)
            nc.gpsimd.iota(row_iota[:, :], pattern=[[1, 128]], base=0,
                           channel_multiplier=0,
                           allow_small_or_imprecise_dtypes=True)
            p_iota = const_pool.tile([128, 1], F32)
            nc.gpsimd.iota(p_iota[:, :], pattern=[[0, 1]], base=0,
                           channel_multiplier=1,
                           allow_small_or_imprecise_dtypes=True)
            ident = const_pool.tile([128, 128], MM_DT)
            nc.vector.tensor_scalar(ident[:, :], row_iota[:, :],
                                    p_iota[:, :], None, OP.is_equal)

            dout = dout_pool.tile([128, G], F32)


            for g in range(G):
                r0 = g * 128
                s_all = stat_pool.tile([128, M], F32)     # sum e per model
                m2 = stat_pool.tile([128, M], F32)        # sum (e-ebar)^2
                negm = stat_pool.tile([128, M], F32)      # -mean e
                ec_tiles = []
                for m in range(M):
                    x_t = x_pool.tile([128, C], F32)
                    nc.sync.dma_start(out=x_t[:, :],
                                      in_=xs[m].ap()[r0:r0 + 128, :])
                    e_t = e_pool.tile([128, C], F32)
                    nc.scalar.activation(e_t[:, :], x_t[:, :], AF.Exp,
                                         scale=1.0 / T,
                                         accum_out=s_all[:, m:m + 1])
                    nc.vector.tensor_scalar(negm[:, m:m + 1],
                                            s_all[:, m:m + 1],
                                            -1.0 / C, None, OP.mult)
                    # centered ec = e - mean(e); the variance MUST be the
                    # sum of squares of the centered values (quadratically
                    # insensitive to the accumulated-S rounding bias; the
                    # fused (e-eb)*e form leaks that bias into alpha)
                    ec_t = ec_pool.tile([128, C], MM_DT)
                    nc.vector.tensor_scalar(ec_t[:, :], e_t[:, :],
                                            negm[:, m:m + 1], None, OP.add)
                    scr = scr_pool.tile([128, C], F32)
                    if m in ACT_VAR_MODELS:
                        nc.scalar.activation(scr[:, :], ec_t[:, :], AF.Square,
                                             accum_out=m2[:, m:m + 1])
                    else:
                        nc.vector.scalar_tensor_tensor(
                            scr[:, :], ec_t[:, :], 0.0, ec_t[:, :],
                            op0=OP.bypass, op1=OP.mult,
                            accum_out=m2[:, m:m + 1])
                    ec_tiles.append(ec_t)

                # alpha = rsqrt(m2) by Newton: y <- y * (1.5 - 0.5 * m2 * y^2)
                alpha = stat_pool.tile([128, M], F32)
                tn = stat_pool.tile([128, M], F32)
                nc.vector.memset(alpha[:, :], RSQRT_SEED)
                for _ in range(NEWTON_ITERS):
                    nc.vector.scalar_tensor_tensor(
                        tn[:, :], alpha[:, :], 0.0, alpha[:, :],
                        op0=OP.bypass, op1=OP.mult)            # y^2
                    nc.vector.scalar_tensor_tensor(
                        tn[:, :], tn[:, :], -0.5, m2[:, :],
                        op0=OP.mult, op1=OP.mult)              # -0.5*y^2*m2
                    nc.vector.scalar_tensor_tensor(
                        alpha[:, :], tn[:, :], 1.5, alpha[:, :],
                        op0=OP.add, op1=OP.mult)               # y*(1.5+that)
                # s = sum_m diag(alpha_m) @ ec_m  (PSUM accumulation)
                s_ps = psum_pool.tile([128, C], F32)
                for m in range(M):
                    w_t = w_pool.tile([128, 128], MM_DT)
                    nc.vector.tensor_scalar(w_t[:, :], ident[:, :],
                                            alpha[:, m:m + 1], None, OP.mult)
                    for c0, c1 in PSUM_SPLITS:
                        nc.tensor.matmul(s_ps[:, c0:c1],
                                         w_t[:, :],
                                         ec_tiles[m][:, c0:c1],
                                         start=(m == 0), stop=(m == M - 1))

                # d = 0.5 * sum s^2 - M/2
                r_col = stat_pool.tile([128, 1], F32)
                scr2 = scr_pool.tile([128, C], F32)
                nc.scalar.activation(scr2[:, :], s_ps[:, :], AF.Square,
                                     accum_out=r_col[:, :])
                nc.vector.tensor_scalar(dout[:, g:g + 1], r_col[:, :],
                                        0.5, -M / 2.0, OP.mult, OP.add)

            nc.sync.dma_start(out=d_dram.ap(), in_=dout[:, :])

    nc.compile()
    return nc


def _get_nc():
    if "nc" not in _cached:
        _cached["nc"] = _build()
    return _cached["nc"]


def kernel(**inputs: np.ndarray) -> np.ndarray:
    nc = _get_nc()
    outs = [np.asarray(inputs[f"outputs{m + 1}"], dtype=np.float32)
            for m in range(M)]
    in_maps = []
    for c in range(N_CORES):
        sl = slice(c * B_SHARD, (c + 1) * B_SHARD)
        in_maps.append(
            {f"x{m}": np.ascontiguousarray(outs[m][sl]) for m in range(M)}
        )
    res = bass_utils.run_bass_kernel_spmd(nc, in_maps,
                                          core_ids=list(range(N_CORES)))
    total = 0.0
    for c in range(N_CORES):
        total += float(res.results[c]["d"].astype(np.float64).sum())
    return np.array(SCALE * total / B, dtype=np.float32)
